# revision 26
# baseline (speedup 1.0000x reference)
"""Causal self-attention (nn_CausalSelfAttention) on 8 TRN2 NeuronCores.

Reference computation (B=2, T=2048, C=1024, H=16 heads, D=64):
    qkv = x @ W_attn.T + b_attn ; split q,k,v
    y   = softmax(causal(q k^T / sqrt(D))) v        (per head)
    out = y @ W_proj.T + b_proj

Sharding: batch (2-way) x head-group (4-way, 4 heads each) -> 8 cores.
Each core computes its batch's attention for its 4 heads plus the partial
c_proj contribution of those heads' channels; the host sums the 4 partials
per batch and adds b_proj once.

Per-core kernel layout (all fp32 storage, float32r matmuls):
    qk^T  [2*CS, T] = wqkT.T @ xT           (transposed so q/k land [D, T])
    v     [T, CS]   = x @ Wv.T              (natural, augmented with ones col)
    per head, per 512-wide query strip, streamed over 128-row key tiles:
        S^T block = k_h qT_h                 -> PSUM [128, 512]
        P^T = exp(S^T / 8)                   -> SBUF (ACT), causal-masked via
                                               affine_select on diagonal blocks
        y^T[65, 512] += v_aug^T P^T          (ones column gives the softmax
                                              denominator in row 64)
        y = y_unnorm * (1/denominator)       (broadcast + DVE mul)
    out partial [T, C] = y^T.T @ wpT         (host adds partials + bias)
"""
import math
from contextlib import ExitStack

import ml_dtypes
import numpy as np

import concourse.bacc as bacc
import concourse.bass as bass
import concourse.mybir as mybir
import concourse.tile as tile
from concourse.bass_utils import run_bass_kernel_spmd

F32 = mybir.dt.float32
F32R = mybir.dt.float32r
BF16 = mybir.dt.bfloat16
MMDT = BF16                    # dtype for all TensorE-facing tensors

N_CORES = 8
B, T, C, H = 2, 2048, 1024, 16
D = 64
GROUPS = N_CORES // B          # head groups per batch = 4
HPC = H // GROUPS              # heads per core = 4
CS = HPC * D                   # channel slice per core = 256


def build_nc(T_=T, C_=C, CS_=CS):
    """Build + compile the per-core Bass program (SPMD: same program, 8 cores)."""
    TT = T_ // 128             # T tiles
    KT = C_ // 128             # contraction tiles over C
    NS = T_ // 512             # 512-wide query strips
    HL = CS_ // D              # heads on this core
    MQK = 2 * CS_ // 128       # m-tiles of the joint q|k channel block
    KP = CS_ // 128            # contraction tiles for the projection

    nc = bacc.Bacc("TRN2", target_bir_lowering=False, debug=False,
                   num_devices=N_CORES)

    xT = nc.dram_tensor("xT", [C_, T_], MMDT, kind="ExternalInput")
    wqkT = nc.dram_tensor("wqkT", [C_, 2 * CS_], MMDT, kind="ExternalInput")
    bqk = nc.dram_tensor("bqk", [MQK, 128, 1], F32, kind="ExternalInput")
    wvT = nc.dram_tensor("wvT", [C_, CS_], MMDT, kind="ExternalInput")
    bv = nc.dram_tensor("bv", [1, (CS_ // D) * (D + 1)], F32,
                        kind="ExternalInput")
    wpT = nc.dram_tensor("wpT", [CS_, C_], MMDT, kind="ExternalInput")
    out = nc.dram_tensor("out", [T_, C_], F32, kind="ExternalOutput")

    xTr = xT.ap().rearrange("(kt p) t -> kt p t", p=128)
    wqkr = wqkT.ap().rearrange("(kt p) n -> kt p n", p=128)
    wvr = wvT.ap().rearrange("(kt p) n -> kt p n", p=128)
    wpr = wpT.ap().rearrange("(kt p) n -> kt p n", p=128)

    scale = 1.0 / math.sqrt(D)

    with tile.TileContext(nc) as tc, ExitStack() as ctx:
        px = ctx.enter_context(tc.tile_pool(name="px", bufs=1))
        pw = ctx.enter_context(tc.tile_pool(name="pw", bufs=1))
        pqk = ctx.enter_context(tc.tile_pool(name="pqk", bufs=1))
        pv = ctx.enter_context(tc.tile_pool(name="pv", bufs=1))
        py = ctx.enter_context(tc.tile_pool(name="py", bufs=1))
        ppt = ctx.enter_context(tc.tile_pool(name="ppt", bufs=8))
        pnorm = ctx.enter_context(tc.tile_pool(name="pnorm", bufs=2))
        pout = ctx.enter_context(tc.tile_pool(name="pout", bufs=3))
        # 4 rotating banks for every short-lived psum tile (qkv/v/proj
        # accumulation groups and the S^T stream); 4 banks for the per-head
        # PV accumulators which live across a whole strip.
        pst = ctx.enter_context(tc.tile_pool(name="pst", bufs=4, space="PSUM"))
        psy = ctx.enter_context(tc.tile_pool(name="psy", bufs=4, space="PSUM"))

        # ---- input DMA ----
        # dma_start costs ~600ns of sequencer time per descriptor; spread the
        # issue across otherwise-idle engines so transfers start early.
        x_sb, wqk_sb, wv_sb = [], [], []
        for k in range(KT):
            xt = px.tile([128, T_], MMDT, tag=f"x{k}", name=f"x{k}")
            h = T_ // 2
            nc.sync.dma_start(xt[:, 0:h], xTr[k][:, 0:h])
            nc.scalar.dma_start(xt[:, h:T_], xTr[k][:, h:T_])
            x_sb.append(xt)
            wt = pw.tile([128, 2 * CS_], MMDT, tag=f"wqk{k}", name=f"wqk{k}")
            nc.gpsimd.dma_start(wt[:], wqkr[k])
            wqk_sb.append(wt)
            vt = pw.tile([128, CS_], MMDT, tag=f"wv{k}", name=f"wv{k}")
            nc.gpsimd.dma_start(vt[:], wvr[k])
            wv_sb.append(vt)
        wp_sb = []
        for k2 in range(KP):
            pt_ = pw.tile([128, C_], MMDT, tag=f"wp{k2}", name=f"wp{k2}")
            nc.sync.dma_start(pt_[:], wpr[k2])
            wp_sb.append(pt_)
        bqk_sb = []
        for m in range(MQK):
            bt = pw.tile([128, 1], F32, tag=f"bqk{m}", name=f"bqk{m}")
            nc.gpsimd.dma_start(bt[:], bqk.ap()[m])
            bqk_sb.append(bt)
        # bv is packed per head as [bias(D), 1.0]; the trailing 1.0 feeds the
        # ones column of v_aug (softmax denominator accumulator).
        bv_row = pw.tile([1, HL * (D + 1)], F32, tag="bv_row", name="bv_row")
        nc.sync.dma_start(bv_row[:], bv.ap())
        bv_bc = pw.tile([128, HL * (D + 1)], F32, tag="bv_bc", name="bv_bc")
        nc.gpsimd.partition_broadcast(bv_bc[:], bv_row[:])

        # ---- phase 1: qk^T [2*CS, T] = wqkT.T @ xT  (+ bias) ----
        qk_sb = []
        for m in range(MQK):
            qt = pqk.tile([128, T_], MMDT, tag=f"qk{m}", name=f"qk{m}")
            qk_sb.append(qt)
        for m in range(MQK):
            for s in range(T_ // 512):
                ps = pst.tile([128, 512], F32, tag="st", name="ps_qk")
                for k in range(KT):
                    nc.tensor.matmul(
                        ps[:],
                        wqk_sb[k][:, m * 128:(m + 1) * 128],
                        x_sb[k][:, s * 512:(s + 1) * 512],
                        start=(k == 0), stop=(k == KT - 1),
                    )
                nc.vector.tensor_scalar_add(
                    qk_sb[m][:, s * 512:(s + 1) * 512], ps[:], bqk_sb[m][:])

        # ---- phase 2: v natural [T, CS] + ones column per head ----
        v_sb = []
        for t in range(TT):
            vt = pv.tile([128, HL * (D + 1)], MMDT, tag=f"v{t}", name=f"v{t}")
            v_sb.append(vt)
        for t in range(TT):
            ps = pst.tile([128, CS_], F32, tag="st", name="ps_v")
            for k in range(KT):
                nc.tensor.matmul(
                    ps[:],
                    x_sb[k][:, t * 128:(t + 1) * 128],
                    wv_sb[k][:],
                    start=(k == 0), stop=(k == KT - 1),
                )
            # v_sb[t][:, h*(D+1) : h*(D+1)+D] = ps[:, h*D:(h+1)*D] + bias
            vgrp = v_sb[t][:].rearrange("p (g e) -> p g e", e=D + 1)
            vsrc = ps[:].rearrange("p (g e) -> p g e", e=D)
            bgrp = bv_bc[:].rearrange("p (g e) -> p g e", e=D + 1)
            nc.vector.tensor_tensor(
                vgrp[:, :, 0:D], vsrc, bgrp[:, :, 0:D], op=mybir.AluOpType.add)
            # ones columns (value 1.0 shipped in bv)
            nc.vector.tensor_copy(vgrp[:, :, D:D + 1], bgrp[:, :, D:D + 1])

        # ---- phase 3: attention per head / strip ----
        y_sb = []
        for k2 in range(KP):
            yt = py.tile([128, T_], MMDT, tag=f"y{k2}", name=f"y{k2}")
            y_sb.append(yt)
        # All HL heads advance together through each key-tile round so the PE
        # sees a long dependency-free matmul stream (4 S^T then 4 PV per
        # round) while ACT exps the previous head's block. Projection for a
        # query strip is emitted as soon as all heads finish that strip.
        CCH = min(512, C_)

        def head_slices(hl):
            lo = (hl % 2) * D
            qh = qk_sb[hl // 2][lo:lo + D, :]
            kh = qk_sb[KP + hl // 2][lo:lo + D, :]
            return lo, qh, kh

        for s in reversed(range(NS)):
            nt = 4 * s + 4
            yps = []
            for hl in range(HL):
                ypt = psy.tile([D + 1, 512], F32, tag="yp", name=f"yp{hl}")
                yps.append(ypt)
            for n in range(nt):
                # diagonal super-tile: columns < off are fully masked --
                # skip them in S^T, exp, and the PV accumulation.
                off = max(0, (n - 4 * s)) * 128
                ptiles = []
                for hl in range(HL):
                    lo, qh, kh = head_slices(hl)
                    st = pst.tile([128, 512], F32, tag="st", name="st")
                    nc.tensor.matmul(
                        st[:, off:512],
                        kh[:, n * 128:(n + 1) * 128],
                        qh[:, s * 512 + off:(s + 1) * 512],
                        start=True, stop=True,
                    )
                    ptile = ppt.tile([128, 512], MMDT, tag="pt", name="ptile")
                    nc.scalar.activation(
                        ptile[:, off:512], st[:, off:512],
                        mybir.ActivationFunctionType.Exp, scale=scale)
                    if n >= 4 * s:
                        # mixed 128-col block: keep where q >= key,
                        # i.e. (off + f) - p >= 0 within the block
                        nc.gpsimd.affine_select(
                            out=ptile[:, off:off + 128],
                            in_=ptile[:, off:off + 128],
                            compare_op=mybir.AluOpType.is_ge,
                            fill=0.0, base=0,
                            pattern=[[1, 128]], channel_multiplier=-1)
                    ptiles.append(ptile)
                for hl in range(HL):
                    nc.tensor.matmul(
                        yps[hl][:, off:512],
                        v_sb[n][:, hl * (D + 1):(hl + 1) * (D + 1)],
                        ptiles[hl][:, off:512],
                        start=(n == 0), stop=(n == nt - 1),
                    )
            for hl in range(HL):
                lo = (hl % 2) * D
                rs = pnorm.tile([1, 512], F32, tag="rs", name="rs")
                nc.vector.tensor_copy(rs[:], yps[hl][D:D + 1, :])
                rr = pnorm.tile([1, 512], F32, tag="rr", name="rr")
                nc.vector.reciprocal_approx_fast(rr[:], rs[:])
                rb = pnorm.tile([D, 512], F32, tag="rb", name="rb")
                nc.gpsimd.partition_broadcast(rb[:], rr[:])
                nc.vector.tensor_tensor(
                    y_sb[(hl * D) // 128][lo:lo + D, s * 512:(s + 1) * 512],
                    yps[hl][0:D, :], rb[:], op=mybir.AluOpType.mult)

            # ---- projection for this strip's four T-tiles ----
            for t in range(4 * s, 4 * s + 4):
                if t >= TT:
                    continue
                for cch in range(C_ // CCH):
                    ps = pst.tile([128, CCH], F32, tag="st", name="ps_o")
                    for k2 in range(KP):
                        nc.tensor.matmul(
                            ps[:],
                            y_sb[k2][:, t * 128:(t + 1) * 128],
                            wp_sb[k2][:, cch * CCH:(cch + 1) * CCH],
                            start=(k2 == 0), stop=(k2 == KP - 1),
                        )
                    ot = pout.tile([128, CCH], F32, tag="ot", name="ot")
                    nc.vector.tensor_copy(ot[:], ps[:])
                    nc.sync.dma_start(
                        out.ap()[t * 128:(t + 1) * 128,
                                 cch * CCH:(cch + 1) * CCH],
                        ot[:])

    nc.compile()
    return nc


def make_in_maps(x, W_attn, b_attn, W_proj):
    """Shard full inputs into the 8 per-core input dicts."""
    x = np.asarray(x, dtype=np.float32)
    W_attn = np.asarray(W_attn, dtype=np.float32)
    b_attn = np.asarray(b_attn, dtype=np.float32)
    W_proj = np.asarray(W_proj, dtype=np.float32)
    Cq = C
    in_maps = []
    xTb = [np.ascontiguousarray(x[b_].T) for b_ in range(B)]
    for core in range(N_CORES):
        b_ = core // GROUPS
        g = core % GROUPS
        sl = slice(g * CS, (g + 1) * CS)
        wq = W_attn[sl, :]
        wk = W_attn[Cq + g * CS:Cq + (g + 1) * CS, :]
        wv = W_attn[2 * Cq + g * CS:2 * Cq + (g + 1) * CS, :]
        bq = b_attn[sl]
        bk = b_attn[Cq + g * CS:Cq + (g + 1) * CS]
        bvs = b_attn[2 * Cq + g * CS:2 * Cq + (g + 1) * CS]
        in_maps.append({
            "xT": xTb[b_].astype(ml_dtypes.bfloat16),
            "wqkT": np.ascontiguousarray(np.concatenate([wq, wk], 0).T).astype(ml_dtypes.bfloat16),
            "bqk": np.ascontiguousarray(
                np.concatenate([bq, bk]).reshape(2 * CS // 128, 128, 1)),
            "wvT": np.ascontiguousarray(wv.T).astype(ml_dtypes.bfloat16),
            "bv": np.ascontiguousarray(
                np.concatenate([bvs.reshape(HPC, D),
                                np.ones((HPC, 1), np.float32)],
                               axis=1).reshape(1, HPC * (D + 1))),
            "wpT": np.ascontiguousarray(W_proj[:, g * CS:(g + 1) * CS].T).astype(ml_dtypes.bfloat16),
        })
    return in_maps


_NC = None


def _get_nc():
    global _NC
    if _NC is None:
        _NC = build_nc()
    return _NC


def run(x, W_attn, b_attn, W_proj, b_proj, trace=False):
    nc = _get_nc()
    in_maps = make_in_maps(x, W_attn, b_attn, W_proj)
    res = run_bass_kernel_spmd(nc, in_maps, core_ids=list(range(N_CORES)),
                               trace=trace)
    out = np.zeros((B, T, C), dtype=np.float32)
    for core in range(N_CORES):
        out[core // GROUPS] += res.results[core]["out"]
    out += np.asarray(b_proj, dtype=np.float32)[None, None, :]
    return out, res


def kernel(x, W_attn, b_attn, W_proj, b_proj):
    out, _ = run(x, W_attn, b_attn, W_proj, b_proj, trace=False)
    return out


# revision 27
# speedup vs baseline: 1.1454x; 1.1454x over previous
"""Causal self-attention (nn_CausalSelfAttention) on 8 TRN2 NeuronCores.

Reference computation (B=2, T=2048, C=1024, H=16 heads, D=64):
    qkv = x @ W_attn.T + b_attn ; split q,k,v
    y   = softmax(causal(q k^T / sqrt(D))) v        (per head)
    out = y @ W_proj.T + b_proj

Sharding: batch (2-way) x head-group (4-way, 4 heads each) -> 8 cores.
Each core computes its batch's attention for its 4 heads plus the partial
c_proj contribution of those heads' channels; the host sums the 4 partials
per batch and adds b_proj once.

Per-core kernel layout (all fp32 storage, float32r matmuls):
    qk^T  [2*CS, T] = wqkT.T @ xT           (transposed so q/k land [D, T])
    v     [T, CS]   = x @ Wv.T              (natural, augmented with ones col)
    per head, per 512-wide query strip, streamed over 128-row key tiles:
        S^T block = k_h qT_h                 -> PSUM [128, 512]
        P^T = exp(S^T / 8)                   -> SBUF (ACT), causal-masked via
                                               affine_select on diagonal blocks
        y^T[65, 512] += v_aug^T P^T          (ones column gives the softmax
                                              denominator in row 64)
        y = y_unnorm * (1/denominator)       (broadcast + DVE mul)
    out partial [T, C] = y^T.T @ wpT         (host adds partials + bias)
"""
import math
from contextlib import ExitStack

import ml_dtypes
import numpy as np

import concourse.bacc as bacc
import concourse.bass as bass
import concourse.mybir as mybir
import concourse.tile as tile
from concourse.bass_utils import run_bass_kernel_spmd

F32 = mybir.dt.float32
F32R = mybir.dt.float32r
BF16 = mybir.dt.bfloat16
MMDT = BF16                    # dtype for all TensorE-facing tensors

N_CORES = 8
B, T, C, H = 2, 2048, 1024, 16
D = 64
GROUPS = N_CORES // B          # head groups per batch = 4
HPC = H // GROUPS              # heads per core = 4
CS = HPC * D                   # channel slice per core = 256


def build_nc(T_=T, C_=C, CS_=CS):
    """Build + compile the per-core Bass program (SPMD: same program, 8 cores)."""
    TT = T_ // 128             # T tiles
    KT = C_ // 128             # contraction tiles over C
    NS = T_ // 512             # 512-wide query strips
    HL = CS_ // D              # heads on this core
    MQK = 2 * CS_ // 128       # m-tiles of the joint q|k channel block
    KP = CS_ // 128            # contraction tiles for the projection

    nc = bacc.Bacc("TRN2", target_bir_lowering=False, debug=False,
                   num_devices=N_CORES)

    xT = nc.dram_tensor("xT", [C_, T_], MMDT, kind="ExternalInput")
    wqkT = nc.dram_tensor("wqkT", [C_, 2 * CS_], MMDT, kind="ExternalInput")
    bqk = nc.dram_tensor("bqk", [MQK, 128, 1], F32, kind="ExternalInput")
    wvT = nc.dram_tensor("wvT", [C_, CS_], MMDT, kind="ExternalInput")
    bv = nc.dram_tensor("bv", [1, (CS_ // D) * (D + 1)], F32,
                        kind="ExternalInput")
    wpT = nc.dram_tensor("wpT", [CS_, C_], MMDT, kind="ExternalInput")
    out = nc.dram_tensor("out", [T_, C_], F32, kind="ExternalOutput")

    xTr = xT.ap().rearrange("(kt p) t -> kt p t", p=128)
    wqkr = wqkT.ap().rearrange("(kt p) n -> kt p n", p=128)
    wvr = wvT.ap().rearrange("(kt p) n -> kt p n", p=128)
    wpr = wpT.ap().rearrange("(kt p) n -> kt p n", p=128)

    scale = 1.0 / math.sqrt(D)

    with tile.TileContext(nc) as tc, ExitStack() as ctx:
        px = ctx.enter_context(tc.tile_pool(name="px", bufs=1))
        pw = ctx.enter_context(tc.tile_pool(name="pw", bufs=1))
        pqk = ctx.enter_context(tc.tile_pool(name="pqk", bufs=1))
        pv = ctx.enter_context(tc.tile_pool(name="pv", bufs=1))
        py = ctx.enter_context(tc.tile_pool(name="py", bufs=1))
        ppt = ctx.enter_context(tc.tile_pool(name="ppt", bufs=8))
        pnorm = ctx.enter_context(tc.tile_pool(name="pnorm", bufs=2))
        pout = ctx.enter_context(tc.tile_pool(name="pout", bufs=3))
        pmm = ctx.enter_context(tc.tile_pool(name="pmm", bufs=1, space="PSUM"))
        pst = ctx.enter_context(tc.tile_pool(name="pst", bufs=3, space="PSUM"))
        psy = ctx.enter_context(tc.tile_pool(name="psy", bufs=4, space="PSUM"))

        # ---- input DMA ----
        # dma_start costs ~600ns of sequencer time per descriptor; spread the
        # issue across otherwise-idle engines so transfers start early.
        x_sb, wqk_sb, wv_sb = [], [], []
        for k in range(KT):
            xt = px.tile([128, T_], MMDT, tag=f"x{k}", name=f"x{k}")
            h = T_ // 2
            nc.sync.dma_start(xt[:, 0:h], xTr[k][:, 0:h])
            nc.scalar.dma_start(xt[:, h:T_], xTr[k][:, h:T_])
            x_sb.append(xt)
            wt = pw.tile([128, 2 * CS_], MMDT, tag=f"wqk{k}", name=f"wqk{k}")
            nc.gpsimd.dma_start(wt[:], wqkr[k])
            wqk_sb.append(wt)
            vt = pw.tile([128, CS_], MMDT, tag=f"wv{k}", name=f"wv{k}")
            nc.gpsimd.dma_start(vt[:], wvr[k])
            wv_sb.append(vt)
        wp_sb = []
        for k2 in range(KP):
            pt_ = pw.tile([128, C_], MMDT, tag=f"wp{k2}", name=f"wp{k2}")
            nc.sync.dma_start(pt_[:], wpr[k2])
            wp_sb.append(pt_)
        bqk_sb = []
        for m in range(MQK):
            bt = pw.tile([128, 1], F32, tag=f"bqk{m}", name=f"bqk{m}")
            nc.gpsimd.dma_start(bt[:], bqk.ap()[m])
            bqk_sb.append(bt)
        # bv is packed per head as [bias(D), 1.0]; the trailing 1.0 feeds the
        # ones column of v_aug (softmax denominator accumulator).
        bv_row = pw.tile([1, HL * (D + 1)], F32, tag="bv_row", name="bv_row")
        nc.sync.dma_start(bv_row[:], bv.ap())
        bv_bc = pw.tile([128, HL * (D + 1)], F32, tag="bv_bc", name="bv_bc")
        nc.gpsimd.partition_broadcast(bv_bc[:], bv_row[:])

        # ---- HAM warm-up: ~5us of dummy matmuls while input DMAs land ----
        warm = pw.tile([128, 512], MMDT, tag="warm", name="warm")
        nc.vector.memset(warm[:], 0.0)
        wps = pst.tile([128, 512], F32, tag="st", name="wps")
        for _ in range(24):
            nc.tensor.matmul(wps[:], warm[:, 0:128], warm[:],
                             start=True, stop=True)

        # ---- phase 1: qk^T [2*CS, T] = wqkT.T @ xT  (+ bias) ----
        qk_sb = []
        for m in range(MQK):
            qt = pqk.tile([128, T_], MMDT, tag=f"qk{m}", name=f"qk{m}")
            qk_sb.append(qt)
        for m in range(MQK):
            for s in range(T_ // 512):
                ps = pmm.tile([128, 512], F32, tag="mm", name="ps_qk")
                for k in range(KT):
                    nc.tensor.matmul(
                        ps[:],
                        wqk_sb[k][:, m * 128:(m + 1) * 128],
                        x_sb[k][:, s * 512:(s + 1) * 512],
                        start=(k == 0), stop=(k == KT - 1),
                    )
                nc.vector.tensor_scalar_add(
                    qk_sb[m][:, s * 512:(s + 1) * 512], ps[:], bqk_sb[m][:])

        # ---- phase 2: v natural [T, CS] + ones column per head ----
        v_sb = []
        for t in range(TT):
            vt = pv.tile([128, HL * (D + 1)], MMDT, tag=f"v{t}", name=f"v{t}")
            v_sb.append(vt)
        for t in range(TT):
            ps = pmm.tile([128, CS_], F32, tag="mm", name="ps_v")
            for k in range(KT):
                nc.tensor.matmul(
                    ps[:],
                    x_sb[k][:, t * 128:(t + 1) * 128],
                    wv_sb[k][:],
                    start=(k == 0), stop=(k == KT - 1),
                )
            # v_sb[t][:, h*(D+1) : h*(D+1)+D] = ps[:, h*D:(h+1)*D] + bias
            vgrp = v_sb[t][:].rearrange("p (g e) -> p g e", e=D + 1)
            vsrc = ps[:].rearrange("p (g e) -> p g e", e=D)
            bgrp = bv_bc[:].rearrange("p (g e) -> p g e", e=D + 1)
            nc.vector.tensor_tensor(
                vgrp[:, :, 0:D], vsrc, bgrp[:, :, 0:D], op=mybir.AluOpType.add)
            # ones columns (value 1.0 shipped in bv)
            nc.vector.tensor_copy(vgrp[:, :, D:D + 1], bgrp[:, :, D:D + 1])

        # ---- phase 3: attention per head / strip ----
        y_sb = []
        for k2 in range(KP):
            yt = py.tile([128, T_], MMDT, tag=f"y{k2}", name=f"y{k2}")
            y_sb.append(yt)
        # All HL heads advance together through each key-tile round so the PE
        # sees a long dependency-free matmul stream (4 S^T then 4 PV per
        # round) while ACT exps the previous head's block. Projection for a
        # query strip is emitted as soon as all heads finish that strip.
        CCH = min(512, C_)

        def head_slices(hl):
            lo = (hl % 2) * D
            qh = qk_sb[hl // 2][lo:lo + D, :]
            kh = qk_sb[KP + hl // 2][lo:lo + D, :]
            return lo, qh, kh

        for s in reversed(range(NS)):
            nt = 4 * s + 4
            yps = []
            for hl in range(HL):
                ypt = psy.tile([D + 1, 512], F32, tag="yp", name=f"yp{hl}")
                yps.append(ypt)
            for n in range(nt):
                # diagonal super-tile: columns < off are fully masked --
                # skip them in S^T, exp, and the PV accumulation.
                off = max(0, (n - 4 * s)) * 128
                ptiles = []
                for hl in range(HL):
                    lo, qh, kh = head_slices(hl)
                    st = pst.tile([128, 512], F32, tag="st", name="st")
                    nc.tensor.matmul(
                        st[:, off:512],
                        kh[:, n * 128:(n + 1) * 128],
                        qh[:, s * 512 + off:(s + 1) * 512],
                        start=True, stop=True,
                    )
                    ptile = ppt.tile([128, 512], MMDT, tag="pt", name="ptile")
                    nc.scalar.activation(
                        ptile[:, off:512], st[:, off:512],
                        mybir.ActivationFunctionType.Exp, scale=scale)
                    if n >= 4 * s:
                        # mixed 128-col block: keep where q >= key,
                        # i.e. (off + f) - p >= 0 within the block
                        nc.gpsimd.affine_select(
                            out=ptile[:, off:off + 128],
                            in_=ptile[:, off:off + 128],
                            compare_op=mybir.AluOpType.is_ge,
                            fill=0.0, base=0,
                            pattern=[[1, 128]], channel_multiplier=-1)
                    ptiles.append(ptile)
                for hl in range(HL):
                    nc.tensor.matmul(
                        yps[hl][:, off:512],
                        v_sb[n][:, hl * (D + 1):(hl + 1) * (D + 1)],
                        ptiles[hl][:, off:512],
                        start=(n == 0), stop=(n == nt - 1),
                    )
            for hl in range(HL):
                lo = (hl % 2) * D
                rs = pnorm.tile([1, 512], F32, tag="rs", name="rs")
                nc.vector.tensor_copy(rs[:], yps[hl][D:D + 1, :])
                rr = pnorm.tile([1, 512], F32, tag="rr", name="rr")
                nc.vector.reciprocal_approx_fast(rr[:], rs[:])
                rb = pnorm.tile([D, 512], F32, tag="rb", name="rb")
                nc.gpsimd.partition_broadcast(rb[:], rr[:])
                nc.vector.tensor_tensor(
                    y_sb[(hl * D) // 128][lo:lo + D, s * 512:(s + 1) * 512],
                    yps[hl][0:D, :], rb[:], op=mybir.AluOpType.mult)

            # ---- projection for this strip's four T-tiles ----
            for t in range(4 * s, 4 * s + 4):
                if t >= TT:
                    continue
                for cch in range(C_ // CCH):
                    ps = pmm.tile([128, CCH], F32, tag="mm", name="ps_o")
                    for k2 in range(KP):
                        nc.tensor.matmul(
                            ps[:],
                            y_sb[k2][:, t * 128:(t + 1) * 128],
                            wp_sb[k2][:, cch * CCH:(cch + 1) * CCH],
                            start=(k2 == 0), stop=(k2 == KP - 1),
                        )
                    ot = pout.tile([128, CCH], F32, tag="ot", name="ot")
                    nc.vector.tensor_copy(ot[:], ps[:])
                    nc.sync.dma_start(
                        out.ap()[t * 128:(t + 1) * 128,
                                 cch * CCH:(cch + 1) * CCH],
                        ot[:])

    nc.compile()
    return nc


def make_in_maps(x, W_attn, b_attn, W_proj):
    """Shard full inputs into the 8 per-core input dicts."""
    x = np.asarray(x, dtype=np.float32)
    W_attn = np.asarray(W_attn, dtype=np.float32)
    b_attn = np.asarray(b_attn, dtype=np.float32)
    W_proj = np.asarray(W_proj, dtype=np.float32)
    Cq = C
    in_maps = []
    xTb = [np.ascontiguousarray(x[b_].T) for b_ in range(B)]
    for core in range(N_CORES):
        b_ = core // GROUPS
        g = core % GROUPS
        sl = slice(g * CS, (g + 1) * CS)
        wq = W_attn[sl, :]
        wk = W_attn[Cq + g * CS:Cq + (g + 1) * CS, :]
        wv = W_attn[2 * Cq + g * CS:2 * Cq + (g + 1) * CS, :]
        bq = b_attn[sl]
        bk = b_attn[Cq + g * CS:Cq + (g + 1) * CS]
        bvs = b_attn[2 * Cq + g * CS:2 * Cq + (g + 1) * CS]
        in_maps.append({
            "xT": xTb[b_].astype(ml_dtypes.bfloat16),
            "wqkT": np.ascontiguousarray(np.concatenate([wq, wk], 0).T).astype(ml_dtypes.bfloat16),
            "bqk": np.ascontiguousarray(
                np.concatenate([bq, bk]).reshape(2 * CS // 128, 128, 1)),
            "wvT": np.ascontiguousarray(wv.T).astype(ml_dtypes.bfloat16),
            "bv": np.ascontiguousarray(
                np.concatenate([bvs.reshape(HPC, D),
                                np.ones((HPC, 1), np.float32)],
                               axis=1).reshape(1, HPC * (D + 1))),
            "wpT": np.ascontiguousarray(W_proj[:, g * CS:(g + 1) * CS].T).astype(ml_dtypes.bfloat16),
        })
    return in_maps


_NC = None


def _get_nc():
    global _NC
    if _NC is None:
        _NC = build_nc()
    return _NC


def run(x, W_attn, b_attn, W_proj, b_proj, trace=False):
    nc = _get_nc()
    in_maps = make_in_maps(x, W_attn, b_attn, W_proj)
    res = run_bass_kernel_spmd(nc, in_maps, core_ids=list(range(N_CORES)),
                               trace=trace)
    out = np.zeros((B, T, C), dtype=np.float32)
    for core in range(N_CORES):
        out[core // GROUPS] += res.results[core]["out"]
    out += np.asarray(b_proj, dtype=np.float32)[None, None, :]
    return out, res


def kernel(x, W_attn, b_attn, W_proj, b_proj):
    out, _ = run(x, W_attn, b_attn, W_proj, b_proj, trace=False)
    return out
